# revision 18
# baseline (speedup 1.0000x reference)
"""MoE router (BatchedRouter) Trainium2 Bass kernel.

reference math (fp32):
    logits = x @ gate_weight.T          # [T, 64]
    probs  = softmax(logits, -1)
    top_vals, top_idx = top_k(probs, 8)
    full_probs = scatter(top_vals / sum(top_vals))   # [T, 64]
    return full_probs, top_idx

Distribution: data-parallel over tokens; 8 cores x 2048 tokens each,
gate weight replicated.  The softmax denominator cancels in the
renormalization, so per token we only need: top-8 of logits, exp of the
logit row, and the row masked to the exact top-8 positions, scaled by
1/sum(top-8 exps).  (No max-shift needed: |logits| < ~6 here, exp is
safe in fp32, and the renormalized ratio is unchanged.)

Layout notes:
 - The PE matmul contracts over the partition dim for both operands, so
   x must arrive d-major.  The host also pre-tiles x per (pair, d-tile)
   into contiguous 512 KB blocks so every x DMA is a single sequential
   DRAM read (strided 4 KB lines measured ~14% slower).  Pure layout
   prep - the HW still streams the same 33.5 MB per core from HBM,
   which is the roofline (~94 us/core).
 - The tiny wT d-tile [128, 64] is the matmul stationary (cheap
   LDWEIGHTS, reused over a 512-token moving pass of xT); the
   experts-on-partitions [64, 512] results are transposed back to
   token-major [128, 64] blocks with PE transpose-mode matmuls against
   an identity.  Token halves map to the two column-group halves of the
   PE array (tile_position (0,0)/(0,64)) so the M=64 matmuls run
   two-at-a-time and don't waste half the array.

Per core program (SPMD, identical on cores 0-7):
  - wT [4096, 64] loaded once into SBUF as [128, 32, 64] (gpsimd queue,
    overlapping the first x DMAs on the sync queue)
  - per 1024-token pair: 32 x-block DMAs [128d, 1024t] (contiguous);
    per d-tile two concurrent matmuls into PSUM halves
    psA[0:64, :] / psB[64:128, :] accumulating over d
  - per pair: 2 ACT copies PSUM->SBUF, 8 PE transposes into one PSUM
    bank, 1 batched ACT copy, 1 batched ACT exp of all 512 logits
  - per 128-token block, all on DVE: Max8; MaxIndex8 straight into the
    output tile (hardware tie-breaking identical to jax top_k: first
    index wins); match_replace marks the exact top-8 positions;
    scalar_tensor_tensor computes masked_exp = (marked==BIG)*exp with
    fused row-sum; reciprocal; tensor_scalar rescale into the output
  - batched per-pair output DMA
"""

import numpy as np
from contextlib import ExitStack

import concourse.bass as bass
import concourse.mybir as mybir
import concourse.tile as tile
from concourse import bacc
from concourse.alu_op_type import AluOpType
from concourse.bass_utils import run_bass_kernel_spmd
from concourse.masks import make_identity

P = 128          # SBUF partitions
E = 64           # experts
K = 8            # top-k
D = 4096         # input dim
T_FULL = 16384   # total tokens
N_CORES = 8
T_CORE = T_FULL // N_CORES
PAIR = 512       # tokens per processing chunk (2 x 256 column-group halves)
HALF = 256
BIG = 1.0e30     # match_replace sentinel; no logit can reach this

F32 = mybir.dt.float32
U32 = mybir.dt.uint32


def chunk_list(t_core):
    # trailing 256-token chunks halve the post-stream epilogue depth
    assert t_core % PAIR == 0
    return [PAIR] * (t_core // PAIR - 1) + [PAIR // 2, PAIR // 2]


def build_nc(t_core=T_CORE, d=D):
    d_tiles = d // P
    chunks = chunk_list(t_core)
    assert d % P == 0
    act = mybir.ActivationFunctionType

    nc = bacc.Bacc(
        "TRN2", target_bir_lowering=False, debug=False, num_devices=N_CORES
    )
    # x, host-pre-tiled: for chunk q, d-tile dt, the [128 d, chunk_q t]
    # tile is one contiguous DRAM block (flattened into 256-wide rows).
    xT = nc.declare_dram_parameter("xT", [t_core * d], F32, isOutput=False)
    wT = nc.declare_dram_parameter("wT", [d, E], F32, isOutput=False)
    probs = nc.declare_dram_parameter("probs", [t_core, E], F32, isOutput=True)
    idx = nc.declare_dram_parameter("idx", [t_core, K], U32, isOutput=True)

    with ExitStack() as ctx:
        tc = ctx.enter_context(tile.TileContext(nc))
        wpool = ctx.enter_context(tc.tile_pool(name="w", bufs=1))
        xpool = ctx.enter_context(tc.tile_pool(name="x", bufs=10))
        pspool = ctx.enter_context(tc.tile_pool(name="ps", bufs=4, space="PSUM"))
        pstpool = ctx.enter_context(tc.tile_pool(name="pst", bufs=2, space="PSUM"))
        lgpool = ctx.enter_context(tc.tile_pool(name="lgp", bufs=2))
        spool = ctx.enter_context(tc.tile_pool(name="s", bufs=3))
        opool = ctx.enter_context(tc.tile_pool(name="o", bufs=2))

        # w + identity go through the gpsimd queue so the sync queue's
        # first instruction is already an x-tile DMA.
        w_sb = wpool.tile([P, d_tiles, E], F32)
        nc.gpsimd.dma_start(out=w_sb[:], in_=wT[:].rearrange("(dt p) e -> p dt e", p=P))
        ident = wpool.tile([P, E], F32)
        make_identity(nc, ident[0:E, :])
        make_identity(nc, ident[E:P, :])

        tok0 = 0
        for q, cw in enumerate(chunks):
            half = cw // 2
            n_sub = cw // P
            psA = pspool.tile([P, half], F32, tag="ps", name=f"psA_{q}")
            psB = pspool.tile([P, half], F32, tag="ps", name=f"psB_{q}")

            for dt in range(d_tiles):
                x_t = xpool.tile([P, cw], F32, tag="x", name=f"x_{q}_{dt}")
                o0 = tok0 * d + dt * P * cw
                nc.sync.dma_start(
                    out=x_t[:],
                    in_=xT[o0:o0 + P * cw].rearrange("(p t) -> p t", p=P),
                )
                nc.tensor.matmul(
                    psA[0:E, :], w_sb[:, dt, :], x_t[:, 0:half],
                    start=(dt == 0), stop=(dt == d_tiles - 1),
                    tile_position=(0, 0),
                )
                nc.tensor.matmul(
                    psB[E:P, :], w_sb[:, dt, :], x_t[:, half:cw],
                    start=(dt == 0), stop=(dt == d_tiles - 1),
                    tile_position=(0, E),
                )

            # logits^T for the chunk: [64 experts, half tokens] per half
            lg_sb = lgpool.tile([P, half], F32, tag="lg", name=f"lg_{q}")
            nc.scalar.activation(lg_sb[0:E, :], psA[0:E, :], act.Copy)
            nc.scalar.activation(lg_sb[E:P, :], psB[E:P, :], act.Copy)

            # transpose to token-major blocks, batched into one PSUM bank
            pst = pstpool.tile([P, n_sub * E], F32, tag="pst", name=f"pst_{q}")
            for s in range(n_sub):
                hf, blk = divmod(s, n_sub // 2)
                lo, hi = (0, E) if hf == 0 else (E, P)
                nc.tensor.transpose(
                    pst[:, s * E:(s + 1) * E],
                    lg_sb[lo:hi, blk * P:(blk + 1) * P],
                    ident[lo:hi, :],
                )
            lg8 = lgpool.tile([P, n_sub * E], F32, tag="lg8", name=f"lg8_{q}")
            nc.scalar.activation(lg8[:], pst[:], act.Copy)
            expf = lgpool.tile([P, n_sub * E], F32, tag="expf", name=f"expf_{q}")
            nc.scalar.activation(expf[:], lg8[:], act.Exp)

            probs_t = opool.tile([P, n_sub * E], F32, tag="probs_t")
            idx_t = opool.tile([P, n_sub * K], U32, tag="idx_t")

            for s in range(n_sub):
                lg_s = lg8[:, s * E:(s + 1) * E]
                top8 = spool.tile([P, K], F32, tag="top8")
                nc.vector.max(top8[:], lg_s)
                nc.vector.max_index(idx_t[:, s * K:(s + 1) * K], top8[:], lg_s)
                marked = spool.tile([P, E], F32, tag="marked")
                nc.vector.match_replace(marked[:], top8[:], lg_s, BIG)
                me = spool.tile([P, E], F32, tag="me")
                s8 = spool.tile([P, 1], F32, tag="s8")
                nc.vector.scalar_tensor_tensor(
                    me[:], marked[:], BIG, expf[:, s * E:(s + 1) * E],
                    op0=AluOpType.is_equal, op1=AluOpType.mult,
                    accum_out=s8[:],
                )
                r = spool.tile([P, 1], F32, tag="r")
                nc.vector.reciprocal(r[:], s8[:])
                nc.vector.tensor_scalar_mul(
                    probs_t[:, s * E:(s + 1) * E], me[:], r[:])

            nc.scalar.dma_start(
                out=probs[tok0:tok0 + cw, :].rearrange("(s p) e -> p s e", p=P),
                in_=probs_t[:].rearrange("p (s e) -> p s e", e=E),
            )
            nc.scalar.dma_start(
                out=idx[tok0:tok0 + cw, :].rearrange("(s p) k -> p s k", p=P),
                in_=idx_t[:].rearrange("p (s k) -> p s k", k=K),
            )
            tok0 += cw
    nc.compile()
    return nc


_NC_CACHE = {}


def _get_nc():
    if "nc" not in _NC_CACHE:
        _NC_CACHE["nc"] = build_nc()
    return _NC_CACHE["nc"]


def tile_x_shard(x_shard):
    """[t_core, d] -> flat pre-tiled [t_core*d/256, 256] (see build_nc)."""
    t_core, d = x_shard.shape
    xc = x_shard.T  # [d, t_core]
    parts = []
    tok0 = 0
    for cw in chunk_list(t_core):
        blk = xc[:, tok0:tok0 + cw]                    # [d, cw]
        blk = blk.reshape(d // P, P, cw)               # [dt, p, t]
        parts.append(blk.reshape(-1))
        tok0 += cw
    return np.ascontiguousarray(np.concatenate(parts))


def make_in_maps(x, gate_weight):
    x = np.asarray(x, dtype=np.float32)
    w = np.asarray(gate_weight, dtype=np.float32)
    wT = np.ascontiguousarray(w.T)
    in_maps = []
    t_core = x.shape[0] // N_CORES
    for c in range(N_CORES):
        in_maps.append(
            {"xT": tile_x_shard(x[c * t_core:(c + 1) * t_core, :]), "wT": wT})
    return in_maps


def gather_results(results):
    probs = np.concatenate([results[c]["probs"] for c in range(N_CORES)], axis=0)
    idx = np.concatenate([results[c]["idx"] for c in range(N_CORES)], axis=0)
    return probs, idx.view(np.int32)


def run(x, gate_weight, **spmd_kwargs):
    res = run_bass_kernel_spmd(
        _get_nc(), make_in_maps(x, gate_weight), list(range(N_CORES)), **spmd_kwargs
    )
    return res


def kernel(x, gate_weight):
    res = run(x, gate_weight)
    return gather_results(res.results)


# revision 19
# speedup vs baseline: 1.1549x; 1.1549x over previous
"""MoE router (BatchedRouter) Trainium2 Bass kernel.

reference math (fp32):
    logits = x @ gate_weight.T          # [T, 64]
    probs  = softmax(logits, -1)
    top_vals, top_idx = top_k(probs, 8)
    full_probs = scatter(top_vals / sum(top_vals))   # [T, 64]
    return full_probs, top_idx

Distribution: data-parallel over tokens; 8 cores x 2048 tokens each,
gate weight replicated.  The softmax denominator cancels in the
renormalization, so per token we only need: top-8 of logits, exp of the
logit row, and the row masked to the exact top-8 positions, scaled by
1/sum(top-8 exps).  (No max-shift needed: |logits| < ~6 here, exp is
safe in fp32, and the renormalized ratio is unchanged.)

Layout notes:
 - The PE matmul contracts over the partition dim for both operands, so
   x must arrive d-major.  The host also pre-tiles x per (pair, d-tile)
   into contiguous 512 KB blocks so every x DMA is a single sequential
   DRAM read (strided 4 KB lines measured ~14% slower).  Pure layout
   prep - the HW still streams the same 33.5 MB per core from HBM,
   which is the roofline (~94 us/core).
 - The tiny wT d-tile [128, 64] is the matmul stationary (cheap
   LDWEIGHTS, reused over a 512-token moving pass of xT); the
   experts-on-partitions [64, 512] results are transposed back to
   token-major [128, 64] blocks with PE transpose-mode matmuls against
   an identity.  Token halves map to the two column-group halves of the
   PE array (tile_position (0,0)/(0,64)) so the M=64 matmuls run
   two-at-a-time and don't waste half the array.

Per core program (SPMD, identical on cores 0-7):
  - wT [4096, 64] loaded once into SBUF as [128, 32, 64] (gpsimd queue,
    overlapping the first x DMAs on the sync queue)
  - per 1024-token pair: 32 x-block DMAs [128d, 1024t] (contiguous);
    per d-tile two concurrent matmuls into PSUM halves
    psA[0:64, :] / psB[64:128, :] accumulating over d
  - per pair: 2 ACT copies PSUM->SBUF, 8 PE transposes into one PSUM
    bank, 1 batched ACT copy, 1 batched ACT exp of all 512 logits
  - per 128-token block, all on DVE: Max8; MaxIndex8 straight into the
    output tile (hardware tie-breaking identical to jax top_k: first
    index wins); match_replace marks the exact top-8 positions;
    scalar_tensor_tensor computes masked_exp = (marked==BIG)*exp with
    fused row-sum; reciprocal; tensor_scalar rescale into the output
  - batched per-pair output DMA
"""

import numpy as np
from contextlib import ExitStack

import concourse.bass as bass
import concourse.mybir as mybir
import concourse.tile as tile
from concourse import bacc
from concourse.alu_op_type import AluOpType
from concourse.bass_utils import run_bass_kernel_spmd
from concourse.masks import make_identity

P = 128          # SBUF partitions
E = 64           # experts
K = 8            # top-k
D = 4096         # input dim
T_FULL = 16384   # total tokens
N_CORES = 8
T_CORE = T_FULL // N_CORES
PAIR = 512       # tokens per processing chunk (2 x 256 column-group halves)
HALF = 256
BIG = 1.0e30     # match_replace sentinel; no logit can reach this

F32 = mybir.dt.float32
U32 = mybir.dt.uint32


def chunk_list(t_core):
    # trailing 256-token chunks halve the post-stream epilogue depth
    assert t_core % PAIR == 0
    return [PAIR] * (t_core // PAIR)


def build_nc(t_core=T_CORE, d=D):
    d_tiles = d // P
    chunks = chunk_list(t_core)
    assert d % P == 0
    act = mybir.ActivationFunctionType

    nc = bacc.Bacc(
        "TRN2", target_bir_lowering=False, debug=False, num_devices=N_CORES
    )
    # x, host-pre-tiled: for chunk q, d-tile dt, the [128 d, chunk_q t]
    # tile is one contiguous DRAM block (flattened into 256-wide rows).
    xT = nc.declare_dram_parameter("xT", [t_core * d], F32, isOutput=False)
    wT = nc.declare_dram_parameter("wT", [d, E], F32, isOutput=False)
    probs = nc.declare_dram_parameter("probs", [t_core, E], F32, isOutput=True)
    idx = nc.declare_dram_parameter("idx", [t_core, K], U32, isOutput=True)

    with ExitStack() as ctx:
        tc = ctx.enter_context(tile.TileContext(nc))
        wpool = ctx.enter_context(tc.tile_pool(name="w", bufs=1))
        xpool = ctx.enter_context(tc.tile_pool(name="x", bufs=10))
        pspool = ctx.enter_context(tc.tile_pool(name="ps", bufs=4, space="PSUM"))
        pstpool = ctx.enter_context(tc.tile_pool(name="pst", bufs=2, space="PSUM"))
        lgpool = ctx.enter_context(tc.tile_pool(name="lgp", bufs=2))
        spool = ctx.enter_context(tc.tile_pool(name="s", bufs=3))
        opool = ctx.enter_context(tc.tile_pool(name="o", bufs=2))

        # w + identity go through the gpsimd queue so the sync queue's
        # first instruction is already an x-tile DMA.
        w_sb = wpool.tile([P, d_tiles, E], F32)
        nc.gpsimd.dma_start(out=w_sb[:], in_=wT[:].rearrange("(dt p) e -> p dt e", p=P))
        ident = wpool.tile([P, E], F32)
        make_identity(nc, ident[0:E, :])
        make_identity(nc, ident[E:P, :])

        tok0 = 0
        for q, cw in enumerate(chunks):
            half = cw // 2
            n_sub = cw // P
            psA = pspool.tile([P, half], F32, tag="ps", name=f"psA_{q}")
            psB = pspool.tile([P, half], F32, tag="ps", name=f"psB_{q}")

            for dt in range(d_tiles):
                x_t = xpool.tile([P, cw], F32, tag="x", name=f"x_{q}_{dt}")
                o0 = tok0 * d + dt * P * cw
                nc.sync.dma_start(
                    out=x_t[:],
                    in_=xT[o0:o0 + P * cw].rearrange("(p t) -> p t", p=P),
                )
                nc.tensor.matmul(
                    psA[0:E, :], w_sb[:, dt, :], x_t[:, 0:half],
                    start=(dt == 0), stop=(dt == d_tiles - 1),
                    tile_position=(0, 0),
                )
                nc.tensor.matmul(
                    psB[E:P, :], w_sb[:, dt, :], x_t[:, half:cw],
                    start=(dt == 0), stop=(dt == d_tiles - 1),
                    tile_position=(0, E),
                )

            # logits^T for the chunk: [64 experts, half tokens] per half
            lg_sb = lgpool.tile([P, half], F32, tag="lg", name=f"lg_{q}")
            nc.scalar.activation(lg_sb[0:E, :], psA[0:E, :], act.Copy)
            nc.scalar.activation(lg_sb[E:P, :], psB[E:P, :], act.Copy)

            # transpose to token-major blocks, batched into one PSUM bank
            pst = pstpool.tile([P, n_sub * E], F32, tag="pst", name=f"pst_{q}")
            for s in range(n_sub):
                hf, blk = divmod(s, n_sub // 2)
                lo, hi = (0, E) if hf == 0 else (E, P)
                nc.tensor.transpose(
                    pst[:, s * E:(s + 1) * E],
                    lg_sb[lo:hi, blk * P:(blk + 1) * P],
                    ident[lo:hi, :],
                )
            lg8 = lgpool.tile([P, n_sub * E], F32, tag="lg8", name=f"lg8_{q}")
            nc.scalar.activation(lg8[:], pst[:], act.Copy)
            expf = lgpool.tile([P, n_sub * E], F32, tag="expf", name=f"expf_{q}")
            nc.scalar.activation(expf[:], lg8[:], act.Exp)

            probs_t = opool.tile([P, n_sub * E], F32, tag="probs_t")
            idx_t = opool.tile([P, n_sub * K], U32, tag="idx_t")

            for s in range(n_sub):
                lg_s = lg8[:, s * E:(s + 1) * E]
                top8 = spool.tile([P, K], F32, tag="top8")
                nc.vector.max(top8[:], lg_s)
                nc.vector.max_index(idx_t[:, s * K:(s + 1) * K], top8[:], lg_s)
                marked = spool.tile([P, E], F32, tag="marked")
                nc.vector.match_replace(marked[:], top8[:], lg_s, BIG)
                me = spool.tile([P, E], F32, tag="me")
                s8 = spool.tile([P, 1], F32, tag="s8")
                nc.vector.scalar_tensor_tensor(
                    me[:], marked[:], BIG, expf[:, s * E:(s + 1) * E],
                    op0=AluOpType.is_equal, op1=AluOpType.mult,
                    accum_out=s8[:],
                )
                r = spool.tile([P, 1], F32, tag="r")
                nc.vector.reciprocal(r[:], s8[:])
                nc.vector.tensor_scalar_mul(
                    probs_t[:, s * E:(s + 1) * E], me[:], r[:])

            nc.scalar.dma_start(
                out=probs[tok0:tok0 + cw, :].rearrange("(s p) e -> p s e", p=P),
                in_=probs_t[:].rearrange("p (s e) -> p s e", e=E),
            )
            nc.scalar.dma_start(
                out=idx[tok0:tok0 + cw, :].rearrange("(s p) k -> p s k", p=P),
                in_=idx_t[:].rearrange("p (s k) -> p s k", k=K),
            )
            tok0 += cw
    nc.compile()
    return nc


_NC_CACHE = {}


def _get_nc():
    if "nc" not in _NC_CACHE:
        _NC_CACHE["nc"] = build_nc()
    return _NC_CACHE["nc"]


def tile_x_shard(x_shard):
    """[t_core, d] -> flat pre-tiled [t_core*d/256, 256] (see build_nc)."""
    t_core, d = x_shard.shape
    xc = x_shard.T  # [d, t_core]
    parts = []
    tok0 = 0
    for cw in chunk_list(t_core):
        blk = xc[:, tok0:tok0 + cw]                    # [d, cw]
        blk = blk.reshape(d // P, P, cw)               # [dt, p, t]
        parts.append(blk.reshape(-1))
        tok0 += cw
    return np.ascontiguousarray(np.concatenate(parts))


def make_in_maps(x, gate_weight):
    x = np.asarray(x, dtype=np.float32)
    w = np.asarray(gate_weight, dtype=np.float32)
    wT = np.ascontiguousarray(w.T)
    in_maps = []
    t_core = x.shape[0] // N_CORES
    for c in range(N_CORES):
        in_maps.append(
            {"xT": tile_x_shard(x[c * t_core:(c + 1) * t_core, :]), "wT": wT})
    return in_maps


def gather_results(results):
    probs = np.concatenate([results[c]["probs"] for c in range(N_CORES)], axis=0)
    idx = np.concatenate([results[c]["idx"] for c in range(N_CORES)], axis=0)
    return probs, idx.view(np.int32)


def run(x, gate_weight, **spmd_kwargs):
    res = run_bass_kernel_spmd(
        _get_nc(), make_in_maps(x, gate_weight), list(range(N_CORES)), **spmd_kwargs
    )
    return res


def kernel(x, gate_weight):
    res = run(x, gate_weight)
    return gather_results(res.results)


# revision 20
# speedup vs baseline: 1.1884x; 1.0290x over previous
"""MoE router (BatchedRouter) Trainium2 Bass kernel.

reference math (fp32):
    logits = x @ gate_weight.T          # [T, 64]
    probs  = softmax(logits, -1)
    top_vals, top_idx = top_k(probs, 8)
    full_probs = scatter(top_vals / sum(top_vals))   # [T, 64]
    return full_probs, top_idx

Distribution: data-parallel over tokens; 8 cores x 2048 tokens each,
gate weight replicated.  The softmax denominator cancels in the
renormalization, so per token we only need: top-8 of logits, exp of the
logit row, and the row masked to the exact top-8 positions, scaled by
1/sum(top-8 exps).  (No max-shift needed: |logits| < ~6 here, exp is
safe in fp32, and the renormalized ratio is unchanged.)

Layout notes:
 - The PE matmul contracts over the partition dim for both operands, so
   x must arrive d-major.  The host also pre-tiles x per (pair, d-tile)
   into contiguous 512 KB blocks so every x DMA is a single sequential
   DRAM read (strided 4 KB lines measured ~14% slower).  Pure layout
   prep - the HW still streams the same 33.5 MB per core from HBM,
   which is the roofline (~94 us/core).
 - The tiny wT d-tile [128, 64] is the matmul stationary (cheap
   LDWEIGHTS, reused over a 512-token moving pass of xT); the
   experts-on-partitions [64, 512] results are transposed back to
   token-major [128, 64] blocks with PE transpose-mode matmuls against
   an identity.  Token halves map to the two column-group halves of the
   PE array (tile_position (0,0)/(0,64)) so the M=64 matmuls run
   two-at-a-time and don't waste half the array.

Per core program (SPMD, identical on cores 0-7):
  - wT [4096, 64] loaded once into SBUF as [128, 32, 64] (gpsimd queue,
    overlapping the first x DMAs on the sync queue)
  - per 1024-token pair: 32 x-block DMAs [128d, 1024t] (contiguous);
    per d-tile two concurrent matmuls into PSUM halves
    psA[0:64, :] / psB[64:128, :] accumulating over d
  - per pair: 2 ACT copies PSUM->SBUF, 8 PE transposes into one PSUM
    bank, 1 batched ACT copy, 1 batched ACT exp of all 512 logits
  - per 128-token block, all on DVE: Max8; MaxIndex8 straight into the
    output tile (hardware tie-breaking identical to jax top_k: first
    index wins); match_replace marks the exact top-8 positions;
    scalar_tensor_tensor computes masked_exp = (marked==BIG)*exp with
    fused row-sum; reciprocal; tensor_scalar rescale into the output
  - batched per-pair output DMA
"""

import numpy as np
from contextlib import ExitStack

import concourse.bass as bass
import concourse.mybir as mybir
import concourse.tile as tile
from concourse import bacc
from concourse.alu_op_type import AluOpType
from concourse.bass_utils import run_bass_kernel_spmd
from concourse.masks import make_identity

P = 128          # SBUF partitions
E = 64           # experts
K = 8            # top-k
D = 4096         # input dim
T_FULL = 16384   # total tokens
N_CORES = 8
T_CORE = T_FULL // N_CORES
PAIR = 512       # tokens per processing chunk (2 x 256 column-group halves)
HALF = 256
BIG = 1.0e30     # match_replace sentinel; no logit can reach this

F32 = mybir.dt.float32
U32 = mybir.dt.uint32


def chunk_list(t_core):
    # trailing 256-token chunks halve the post-stream epilogue depth
    assert t_core % PAIR == 0
    return [PAIR] * (t_core // PAIR)


def build_nc(t_core=T_CORE, d=D):
    d_tiles = d // P
    chunks = chunk_list(t_core)
    assert d % P == 0
    act = mybir.ActivationFunctionType

    nc = bacc.Bacc(
        "TRN2", target_bir_lowering=False, debug=False, num_devices=N_CORES
    )
    # x, host-pre-tiled: for chunk q, d-tile dt, the [128 d, chunk_q t]
    # tile is one contiguous DRAM block (flattened into 256-wide rows).
    xT = nc.declare_dram_parameter("xT", [t_core * d], F32, isOutput=False)
    wT = nc.declare_dram_parameter("wT", [d, E], F32, isOutput=False)
    probs = nc.declare_dram_parameter("probs", [t_core, E], F32, isOutput=True)
    idx = nc.declare_dram_parameter("idx", [t_core, K], U32, isOutput=True)

    with ExitStack() as ctx:
        tc = ctx.enter_context(tile.TileContext(nc))
        wpool = ctx.enter_context(tc.tile_pool(name="w", bufs=1))
        xpool = ctx.enter_context(tc.tile_pool(name="x", bufs=16))
        pspool = ctx.enter_context(tc.tile_pool(name="ps", bufs=4, space="PSUM"))
        pstpool = ctx.enter_context(tc.tile_pool(name="pst", bufs=2, space="PSUM"))
        lgpool = ctx.enter_context(tc.tile_pool(name="lgp", bufs=2))
        spool = ctx.enter_context(tc.tile_pool(name="s", bufs=3))
        opool = ctx.enter_context(tc.tile_pool(name="o", bufs=2))

        # w + identity go through the gpsimd queue so the sync queue's
        # first instruction is already an x-tile DMA.
        w_sb = wpool.tile([P, d_tiles, E], F32)
        nc.gpsimd.dma_start(out=w_sb[:], in_=wT[:].rearrange("(dt p) e -> p dt e", p=P))
        ident = wpool.tile([P, E], F32)
        make_identity(nc, ident[0:E, :])
        make_identity(nc, ident[E:P, :])

        tok0 = 0
        for q, cw in enumerate(chunks):
            half = cw // 2
            n_sub = cw // P
            psA = pspool.tile([P, half], F32, tag="ps", name=f"psA_{q}")
            psB = pspool.tile([P, half], F32, tag="ps", name=f"psB_{q}")

            for dt in range(d_tiles):
                x_t = xpool.tile([P, cw], F32, tag="x", name=f"x_{q}_{dt}")
                o0 = tok0 * d + dt * P * cw
                nc.sync.dma_start(
                    out=x_t[:],
                    in_=xT[o0:o0 + P * cw].rearrange("(p t) -> p t", p=P),
                )
                nc.tensor.matmul(
                    psA[0:E, :], w_sb[:, dt, :], x_t[:, 0:half],
                    start=(dt == 0), stop=(dt == d_tiles - 1),
                    tile_position=(0, 0),
                )
                nc.tensor.matmul(
                    psB[E:P, :], w_sb[:, dt, :], x_t[:, half:cw],
                    start=(dt == 0), stop=(dt == d_tiles - 1),
                    tile_position=(0, E),
                )

            # logits^T for the chunk: [64 experts, half tokens] per half
            lg_sb = lgpool.tile([P, half], F32, tag="lg", name=f"lg_{q}")
            nc.scalar.activation(lg_sb[0:E, :], psA[0:E, :], act.Copy)
            nc.scalar.activation(lg_sb[E:P, :], psB[E:P, :], act.Copy)

            # transpose to token-major blocks, batched into one PSUM bank
            pst = pstpool.tile([P, n_sub * E], F32, tag="pst", name=f"pst_{q}")
            for s in range(n_sub):
                hf, blk = divmod(s, n_sub // 2)
                lo, hi = (0, E) if hf == 0 else (E, P)
                nc.tensor.transpose(
                    pst[:, s * E:(s + 1) * E],
                    lg_sb[lo:hi, blk * P:(blk + 1) * P],
                    ident[lo:hi, :],
                )
            lg8 = lgpool.tile([P, n_sub * E], F32, tag="lg8", name=f"lg8_{q}")
            nc.scalar.activation(lg8[:], pst[:], act.Copy)
            expf = lgpool.tile([P, n_sub * E], F32, tag="expf", name=f"expf_{q}")
            nc.scalar.activation(expf[:], lg8[:], act.Exp)

            probs_t = opool.tile([P, n_sub * E], F32, tag="probs_t")
            idx_t = opool.tile([P, n_sub * K], U32, tag="idx_t")

            for s in range(n_sub):
                lg_s = lg8[:, s * E:(s + 1) * E]
                top8 = spool.tile([P, K], F32, tag="top8")
                nc.vector.max(top8[:], lg_s)
                nc.vector.max_index(idx_t[:, s * K:(s + 1) * K], top8[:], lg_s)
                marked = spool.tile([P, E], F32, tag="marked")
                nc.vector.match_replace(marked[:], top8[:], lg_s, BIG)
                me = spool.tile([P, E], F32, tag="me")
                s8 = spool.tile([P, 1], F32, tag="s8")
                nc.vector.scalar_tensor_tensor(
                    me[:], marked[:], BIG, expf[:, s * E:(s + 1) * E],
                    op0=AluOpType.is_equal, op1=AluOpType.mult,
                    accum_out=s8[:],
                )
                r = spool.tile([P, 1], F32, tag="r")
                nc.vector.reciprocal(r[:], s8[:])
                nc.vector.tensor_scalar_mul(
                    probs_t[:, s * E:(s + 1) * E], me[:], r[:])

            nc.scalar.dma_start(
                out=probs[tok0:tok0 + cw, :].rearrange("(s p) e -> p s e", p=P),
                in_=probs_t[:].rearrange("p (s e) -> p s e", e=E),
            )
            nc.scalar.dma_start(
                out=idx[tok0:tok0 + cw, :].rearrange("(s p) k -> p s k", p=P),
                in_=idx_t[:].rearrange("p (s k) -> p s k", k=K),
            )
            tok0 += cw
    nc.compile()
    return nc


_NC_CACHE = {}


def _get_nc():
    if "nc" not in _NC_CACHE:
        _NC_CACHE["nc"] = build_nc()
    return _NC_CACHE["nc"]


def tile_x_shard(x_shard):
    """[t_core, d] -> flat pre-tiled [t_core*d/256, 256] (see build_nc)."""
    t_core, d = x_shard.shape
    xc = x_shard.T  # [d, t_core]
    parts = []
    tok0 = 0
    for cw in chunk_list(t_core):
        blk = xc[:, tok0:tok0 + cw]                    # [d, cw]
        blk = blk.reshape(d // P, P, cw)               # [dt, p, t]
        parts.append(blk.reshape(-1))
        tok0 += cw
    return np.ascontiguousarray(np.concatenate(parts))


def make_in_maps(x, gate_weight):
    x = np.asarray(x, dtype=np.float32)
    w = np.asarray(gate_weight, dtype=np.float32)
    wT = np.ascontiguousarray(w.T)
    in_maps = []
    t_core = x.shape[0] // N_CORES
    for c in range(N_CORES):
        in_maps.append(
            {"xT": tile_x_shard(x[c * t_core:(c + 1) * t_core, :]), "wT": wT})
    return in_maps


def gather_results(results):
    probs = np.concatenate([results[c]["probs"] for c in range(N_CORES)], axis=0)
    idx = np.concatenate([results[c]["idx"] for c in range(N_CORES)], axis=0)
    return probs, idx.view(np.int32)


def run(x, gate_weight, **spmd_kwargs):
    res = run_bass_kernel_spmd(
        _get_nc(), make_in_maps(x, gate_weight), list(range(N_CORES)), **spmd_kwargs
    )
    return res


def kernel(x, gate_weight):
    res = run(x, gate_weight)
    return gather_results(res.results)
